# revision 17
# baseline (speedup 1.0000x reference)
"""AttentiveReadout pooling kernel for 8 Trainium2 NeuronCores.

Math: softmax is shift-invariant, so the reference's clamped segment-max
cancels exactly:
    out[g] = sum_{i in g} x_i * exp(s_i) / sum_{i in g} exp(s_i)
with s_i = tanh(x_i @ W1 + b1) @ w2. Scores are O(1) so exp is safe in f32
without the max subtraction. Both the numerator E[g] = sum w_i x_i and the
denominator S[g] = sum w_i are ADDITIVE over any partition of the nodes, so
nodes can be split into equal chunks with no regard for segment boundaries;
partial (E, S) for segments straddling a boundary are summed on the host.

Sharding: `batch` is sorted. Core d owns nodes [d*PC, (d+1)*PC), split into
NWIN equal windows of WC real nodes padded to C slots on the 256-node grid
(N=500000 -> NWIN=5, WC=12500, C=12544: 0.35% pad) — every window of every
core runs the identical (SPMD) program with identical counts; there is no
data-dependent padding (the old segment-aligned scheme padded every window
to the global max count, +4.6% work). A window's nodes span <= 128
segments (checked; NWIN grows if ever violated), so per 128-node tile the
device builds a one-hot oh[p, seg] = (localseg[p] == seg) * exp(s_p) and
accumulates E[seg, 0:256] += oh.T @ x in PSUM across the window. Full
16-tile groups run 4-tile subs; the 2-tile tail group runs 2-tile subs
(which also shortens the final serial drain chain). The denominator is
computed on the host from the exported exp-weights (rounded through bf16
exactly as the one-hot used them), then out = E / S.

Per-op structure (at full DVFS clocks, per 512-node sub): MLP 4 matmuls x
512 cols (860ns) + 8 score matmuls (216ns) + 4 scatter matmuls x 256 cols
(428ns) on PE; tanh [128,1024] (1.1us) + exp on ACT; 4 one-hot
tensor_scalars on DVE (~310ns each). Steady state is ~99% tensor-busy at
the bf16 streaming floor (MLP 4 cyc/node + scatter 2 cyc/node); DMA
active ~160us (x1 bf16 + xT fp8 e3m4) fits underneath. Scatters run ~6
subs behind their exp so one-hot delivery from DVE never paces the PE.
Measured ~205us (baseline 252us) on fast-clock runs; chip DVFS state
varies run-to-run by up to ~20%.

Key dtype choices:
  - xT (score-MLP copy of x) is fp8 e3m4; W1 stays bf16 (mixed-operand
    matmul runs at full 1 cyc/row). Halves that stream's DMA. Measured
    end-to-end rel-err 0.90% vs the 2e-2 budget (e4m3 would be 2.7%).
  - x1 (scatter copy) stays bf16: scatter feeds the output directly.
  - exp weights round through bf16 in the one-hot; the host denominator
    uses the identical bf16 values, so no extra normalization error.
"""

import os
from contextlib import ExitStack

import ml_dtypes
import numpy as np

import concourse.bass as bass
import concourse.tile as tile
from concourse import bacc, mybir
from concourse.bass_utils import run_bass_kernel_spmd

NCORES = 8
G = 4096
H = 256
P = 128
F32 = mybir.dt.float32
BF16 = mybir.dt.bfloat16
FP8 = mybir.dt.float8e3
TANH = mybir.ActivationFunctionType.Tanh
EXP = mybir.ActivationFunctionType.Exp
BF = ml_dtypes.bfloat16
E3M4 = ml_dtypes.float8_e3m4

_prog_cache: dict[tuple, object] = {}
last_exec_time_ns = None
last_results = None


def _build_program(C: int, NWIN: int, b1_zero: bool = True):
    CT = C // P            # node tiles per window (multiple of 4)
    NG = -(-CT // 16)      # 2048-node DMA groups per window (last may be partial)
    nc = bacc.Bacc("TRN2")

    # x1[w, p, gt*256 + f] = x[node gt*128+p, f]   (node-major, bf16)
    x1 = nc.declare_dram_parameter("x1", [NWIN, P, CT * 256], BF16, isOutput=False)
    # xT[w, k, p, c] = x[node c, k*128+p]          (feature-major, fp8 e3m4)
    xT = nc.declare_dram_parameter("xT", [NWIN, 2, P, C], FP8, isOutput=False)
    # sbt[p, w*CT + gt] = local segment id of node gt*128+p in window w
    sbt = nc.declare_dram_parameter("sbt", [P, NWIN * CT], F32, isOutput=False)
    w1 = nc.declare_dram_parameter("w1", [2, P, 256], BF16, isOutput=False)
    w2b = nc.declare_dram_parameter("w2b", [P, 2], BF16, isOutput=False)
    b1b = nc.declare_dram_parameter("b1b", [P, 2], F32, isOutput=False)
    outp = nc.declare_dram_parameter("out", [NWIN, P, 256], F32, isOutput=True)
    wout = nc.declare_dram_parameter("wout", [NWIN, P, CT], F32, isOutput=True)

    with tile.TileContext(nc) as tc, ExitStack() as ctx:
        cpool = ctx.enter_context(tc.tile_pool(name="consts", bufs=1))
        xpool = ctx.enter_context(tc.tile_pool(name="xblk", bufs=7))
        xtpool = ctx.enter_context(tc.tile_pool(name="xtblk", bufs=6))
        h_ps = ctx.enter_context(tc.tile_pool(name="h_ps", bufs=3, space="PSUM"))
        e_ps = ctx.enter_context(tc.tile_pool(name="e_ps", bufs=1, space="PSUM"))
        w_ps = ctx.enter_context(tc.tile_pool(name="w_ps", bufs=1, space="PSUM"))
        h_sb = ctx.enter_context(tc.tile_pool(name="h_sb", bufs=4))
        wspool = ctx.enter_context(tc.tile_pool(name="wsw", bufs=3))
        ohpool = ctx.enter_context(tc.tile_pool(name="oh", bufs=36))
        opool = ctx.enter_context(tc.tile_pool(name="osb", bufs=2))

        w1t = cpool.tile([P, 2, 256], BF16, name="w1t")
        w2t = cpool.tile([P, 2], BF16, name="w2t")
        b1t = cpool.tile([P, 2], F32, name="b1t")
        iota = cpool.tile([P, P], BF16, name="iota")
        swt = cpool.tile([P, NWIN * CT], F32, name="swt")

        # on-device iota row (0..127 along free dim, same in every partition);
        # 0..127 are exact in bf16
        nc.gpsimd.iota(out=iota[:], pattern=[[1, P]], base=0,
                       channel_multiplier=0,
                       allow_small_or_imprecise_dtypes=True)

        def emit_const_dmas():
            # issued AFTER the first lead xts DMA so the MLP-critical input
            # leads the DMA queue; w1 next (the first LDWEIGHTS needs it)
            nc.sync.dma_start(out=w1t[:], in_=w1[:, :, :])

        def emit_const_dmas2():
            nc.sync.dma_start(out=w2t[:], in_=w2b[:])
            if not b1_zero:
                nc.sync.dma_start(out=b1t[:], in_=b1b[:])
            nc.sync.dma_start(out=swt[:], in_=sbt[:])

        # single PSUM tiles; groups/windows rotate through column halves so
        # WAR tracking provides double-buffering without extra banks
        wp_all = w_ps.tile([P, 32], F32, name="wp_all")
        ept_all = e_ps.tile([P, 512], F32, name="ept_all")

        wstate = {}

        def emit_score(ent):
            """score matmuls for one sub -> wp_all[:, goff + coff + j]."""
            hs, goff = ent["hs"], ent["goff"]
            hw = ent["hw"]
            for j in range(ent["ntiles"]):
                c = goff + ent["coff"] + j
                for m in range(2):
                    nc.tensor.matmul(out=wp_all[:, c: c + 1],
                                     lhsT=hs[:, m * hw + j * 128: m * hw + (j + 1) * 128],
                                     rhs=w2t[:, m: m + 1],
                                     start=(m == 0), stop=(m == 1),
                                     skip_group_check=True)

        def emit_oh(ent):
            """one-hot builds for one sub — emitted early on DVE."""
            w = ent["w"]
            st = wstate[w]
            ohs = []
            for j in range(ent["ntiles"]):
                gt = ent["gt0"] + j
                oh = ohpool.tile([P, P], BF16, name="oh")
                nc.vector.tensor_scalar(
                    out=oh[:], in0=iota[:],
                    scalar1=swt[:, w * CT + gt: w * CT + gt + 1],
                    scalar2=st["wsw"][:, gt: gt + 1],
                    op0=mybir.AluOpType.is_equal, op1=mybir.AluOpType.mult,
                )
                ohs.append(oh)
            ent["ohs"] = ohs

        def emit_scatter(ent):
            """scatter matmuls for one sub (4 tiles)."""
            w = ent["w"]
            x1t, xoff = ent["x1t"], ent["xoff"]
            woff = (w % 2) * 256
            for j in range(ent["ntiles"]):
                gt = ent["gt0"] + j
                nc.tensor.matmul(out=ept_all[:, woff:woff + 256],
                                 lhsT=ent["ohs"][j][:],
                                 rhs=x1t[:, (xoff + j) * 256:(xoff + j + 1) * 256],
                                 start=(gt == 0), stop=(gt == CT - 1),
                                 skip_group_check=True)
            if ent["last"]:
                ot = opool.tile([P, 256], F32, name="ot")
                nc.scalar.copy(ot[:], ept_all[:, woff:woff + 256])
                nc.sync.dma_start(out=outp[w], in_=ot[:])

        pend_score = []    # subs with hs written, score not yet emitted
        pend_exp = []      # blocks (1-2 subs) awaiting exp emission
        pend_scat = []     # subs whose exp is emitted, scatter not yet emitted

        def emit_exp(half_ent):
            """exp over one (1- or 2-sub) block: wp_all slice -> wsw slice."""
            w, goff = half_ent["w"], half_ent["goff"]
            subs = half_ent["subs"]
            wd = sum(s["ntiles"] for s in subs)
            col0 = subs[0]["gt0"]               # wsw column of first tile
            pcol0 = goff + subs[0]["coff"]      # wp_all column
            st = wstate[w]
            nc.scalar.activation(
                out=st["wsw"][:, col0:col0 + wd],
                in_=wp_all[:, pcol0: pcol0 + wd], func=EXP)
            st["nexp"] += wd
            if st["nexp"] >= CT:
                # all of this window's weights are final: export now (tail)
                nc.sync.dma_start(out=wout[w], in_=st["wsw"][:])
            for s in subs:
                emit_oh(s)
            pend_scat.extend(subs)

        for w in range(NWIN):
            wsw = wspool.tile([P, CT], F32, name="wsw")
            wstate[w] = {"wsw": wsw, "nexp": 0}
            for g in range(NG):
                nt = min(16, CT - g * 16)        # tiles in this group
                ns = nt // 4                     # subs in this group
                xts = xtpool.tile([P, 2, 2048], FP8, name="xts")
                if w == 0 and g == 0:
                    # lead chunk: first sub's nodes for both k halves, so the
                    # first MLP matmul can start as early as possible
                    nc.sync.dma_start(out=xts[:, :, :512],
                                      in_=xT[0, :, :, 0:512])
                    emit_const_dmas()
                    nc.sync.dma_start(out=xts[:, :, 512:nt * 128],
                                      in_=xT[0, :, :, 512:nt * 128])
                else:
                    nc.sync.dma_start(
                        out=xts[:, :, :nt * 128],
                        in_=xT[w, :, :, g * 2048:g * 2048 + nt * 128])
                x1t = xpool.tile([P, 16 * 256], BF16, name="x1t")
                nc.sync.dma_start(
                    out=x1t[:, :nt * 256],
                    in_=x1[w, :, g * 16 * 256:g * 16 * 256 + nt * 256])
                if w == 0 and g == 0:
                    emit_const_dmas2()
                goff = ((w * NG + g) % 2) * 16
                last_grp = (w == NWIN - 1 and g == NG - 1)
                subw = 2 if (last_grp or nt % 4 != 0) else 4  # tiles per sub
                nsub = nt // subw
                for subg in range(nsub):
                    toff = subg * subw           # tile offset within group
                    # start a new exp block on 8-tile boundaries; narrow
                    # (tail) subs each get their own block so the final
                    # serial chain is short
                    if subw == 2 or subg % 2 == 0:
                        half_ent = {"w": w, "goff": goff, "subs": []}
                        pend_exp.append(half_ent)
                        cur_ent = half_ent
                    else:
                        cur_ent = pend_exp[-1]
                    cw = subw * 128              # moving columns for this sub
                    hp = h_ps.tile([P, 2 * cw], F32, name="hp")
                    for m in range(2):
                        for k in range(2):
                            nc.tensor.matmul(
                                out=hp[:, m * cw:(m + 1) * cw],
                                lhsT=w1t[:, k, m * 128:(m + 1) * 128],
                                rhs=xts[:, k, toff * 128:toff * 128 + cw],
                                start=(k == 0), stop=(k == 1),
                            )
                    hs = h_sb.tile([P, 2 * cw], BF16, name="hs")
                    if b1_zero:
                        nc.scalar.activation(out=hs[:], in_=hp[:], func=TANH)
                    else:
                        for m in range(2):
                            nc.scalar.activation(
                                out=hs[:, m * cw:(m + 1) * cw],
                                in_=hp[:, m * cw:(m + 1) * cw],
                                func=TANH, bias=b1t[:, m: m + 1])
                    gt0 = g * 16 + toff
                    sent = {"hs": hs, "goff": goff, "coff": toff, "w": w,
                            "gt0": gt0, "ntiles": subw, "hw": cw,
                            "x1t": x1t, "xoff": toff,
                            "last": gt0 + subw == CT}
                    cur_ent["subs"].append(sent)
                    pend_score.append(sent)
                    if len(pend_score) >= 3:
                        emit_score(pend_score.pop(0))
                    # exp(block) once both its subs' scores are emitted
                    if pend_exp and all(
                            s not in pend_score for s in pend_exp[0]["subs"]):
                        emit_exp(pend_exp.pop(0))
                    lim = 3 if (w == NWIN - 1 and g >= NG - 2) else 6
                    if len(pend_scat) >= 5:
                        emit_scatter(pend_scat.pop(0))
                    while len(pend_scat) > lim:
                        emit_scatter(pend_scat.pop(0))
        while pend_score:
            emit_score(pend_score.pop(0))
        while pend_exp:
            emit_exp(pend_exp.pop(0))
        while pend_scat:
            emit_scatter(pend_scat.pop(0))
    nc.finalize()
    return nc


def _get_program(C: int, NWIN: int, b1_zero: bool = True):
    key = (C, NWIN, b1_zero)
    if key not in _prog_cache:
        _prog_cache[key] = _build_program(C, NWIN, b1_zero)
    return _prog_cache[key]


def kernel(x, batch, W1, b1, w2):
    global last_exec_time_ns, last_results
    x = np.ascontiguousarray(np.asarray(x, dtype=np.float32))
    batch_np = np.asarray(batch)
    batch_i = batch_np.astype(np.int64)
    W1 = np.asarray(W1, dtype=np.float32)
    b1 = np.asarray(b1, dtype=np.float32)
    w2 = np.asarray(w2, dtype=np.float32)
    N = x.shape[0]

    PC = -(-N // NCORES)               # nodes per core
    # pick NWIN so every equal-node window spans <= 128 segments
    NWIN = 4
    while True:
        WC = -(-PC // NWIN)            # real nodes per window
        ok = True
        for d in range(NCORES):
            for wloc in range(NWIN):
                ws = min(d * PC + wloc * WC, N)
                we = min(ws + WC, min((d + 1) * PC, N))
                if we > ws and int(batch_i[we - 1] - batch_i[ws]) >= P:
                    ok = False
                    break
            if not ok:
                break
        if ok:
            break
        NWIN += 1
    C = -(-WC // 256) * 256        # CT even; tail group uses 2-tile subs
    CT = C // P
    b1_zero = not bool(np.any(b1))
    nc = _get_program(C, NWIN, b1_zero=b1_zero)

    w1_dev = np.ascontiguousarray(W1.reshape(2, P, 256)).astype(BF)
    w2_dev = np.zeros((P, 2), np.float32)
    w2_dev[:, 0] = w2[:P, 0]
    w2_dev[:, 1] = w2[P:, 0]
    w2_dev = w2_dev.astype(BF)
    b1_dev = np.zeros((P, 2), np.float32)
    b1_dev[:, 0] = b1[:P]
    b1_dev[:, 1] = b1[P:]

    xbf = x.astype(BF)
    x8 = x.astype(E3M4)
    bases = np.zeros((NCORES, NWIN), np.int64)
    in_maps = []
    for d in range(NCORES):
        x1_dev = np.zeros((NWIN, P, CT * 256), BF)
        xT_dev = np.zeros((NWIN, 2, P, C), E3M4)
        sb_dev = np.empty((P, NWIN * CT), np.float32)
        for wloc in range(NWIN):
            ns = min(d * PC + wloc * WC, N)
            ne = min(ns + WC, min((d + 1) * PC, N))
            cnt = max(0, ne - ns)
            svals = np.full(C, -1.0, np.float32)
            if cnt:
                base = int(batch_i[ns])
                bases[d, wloc] = base
                xw = np.zeros((C, 256), BF)
                xw[:cnt] = xbf[ns:ne]
                x1_dev[wloc] = (
                    xw.reshape(CT, P, 256).transpose(1, 0, 2).reshape(P, CT * 256))
                xT_dev[wloc, 0, :, :cnt] = x8[ns:ne, :P].T
                xT_dev[wloc, 1, :, :cnt] = x8[ns:ne, P:].T
                svals[:cnt] = (batch_i[ns:ne] - base).astype(np.float32)
            sb_dev[:, wloc * CT:(wloc + 1) * CT] = svals.reshape(CT, P).T
        in_maps.append({"x1": x1_dev, "xT": xT_dev, "sbt": sb_dev, "w1": w1_dev,
                        "w2b": w2_dev, "b1b": b1_dev})

    res = run_bass_kernel_spmd(nc, in_maps, core_ids=list(range(NCORES)),
                               trace=bool(os.environ.get("KBENCH_TRACE")))
    last_exec_time_ns = res.exec_time_ns
    last_results = res

    E = np.zeros((G, H), np.float32)
    S = np.zeros((G,), np.float64)
    for d in range(NCORES):
        o = res.results[d]["out"]
        wso = res.results[d]["wout"]
        for wloc in range(NWIN):
            ns = min(d * PC + wloc * WC, N)
            ne = min(ns + WC, min((d + 1) * PC, N))
            cnt = max(0, ne - ns)
            if not cnt:
                continue
            base = int(bases[d, wloc])
            rmax = min(P, G - base)
            E[base:base + rmax] += o[wloc][:rmax]
            # node-order weights, rounded through bf16 exactly as the
            # scatter's bf16 one-hot used them
            wnode = (np.asarray(wso[wloc]).astype(np.float32)
                     .astype(BF).astype(np.float64).T.reshape(-1)[:cnt])
            segs = (batch_i[ns:ne] - base).astype(np.int64)
            S[base:base + rmax] += np.bincount(
                segs, weights=wnode, minlength=P)[:rmax]
    S = S.astype(np.float32)
    Ssafe = np.where(S == 0.0, 1.0, S)
    out = np.where((S > 0.0)[:, None], E / Ssafe[:, None], 0.0).astype(np.float32)
    return out


# revision 18
# speedup vs baseline: 1.0202x; 1.0202x over previous
"""AttentiveReadout pooling kernel for 8 Trainium2 NeuronCores.

Math: softmax is shift-invariant, so the reference's clamped segment-max
cancels exactly:
    out[g] = sum_{i in g} x_i * exp(s_i) / sum_{i in g} exp(s_i)
with s_i = tanh(x_i @ W1 + b1) @ w2. Scores are O(1) so exp is safe in f32
without the max subtraction. Both the numerator E[g] = sum w_i x_i and the
denominator S[g] = sum w_i are ADDITIVE over any partition of the nodes, so
nodes can be split into equal chunks with no regard for segment boundaries;
partial (E, S) for segments straddling a boundary are summed on the host.

Sharding: `batch` is sorted. Core d owns nodes [d*PC, (d+1)*PC), split into
NWIN equal windows of WC real nodes padded to C slots on the 256-node grid
(N=500000 -> NWIN=5, WC=12500, C=12544: 0.35% pad) — every window of every
core runs the identical (SPMD) program with identical counts; there is no
data-dependent padding (the old segment-aligned scheme padded every window
to the global max count, +4.6% work). A window's nodes span <= 128
segments (checked; NWIN grows if ever violated), so per 128-node tile the
device builds a one-hot oh[p, seg] = (localseg[p] == seg) * exp(s_p) and
accumulates E[seg, 0:256] += oh.T @ x in PSUM across the window. Full
16-tile groups run 4-tile subs; the 2-tile tail group runs 2-tile subs
(which also shortens the final serial drain chain). The denominator is
computed on the host from the exported exp-weights (rounded through bf16
exactly as the one-hot used them), then out = E / S.

Per-op structure (at full DVFS clocks, per 512-node sub): MLP 4 matmuls x
512 cols (860ns) + 8 score matmuls (216ns) + 4 scatter matmuls x 256 cols
(428ns) on PE; tanh [128,1024] (1.1us) + exp on ACT; 4 one-hot
tensor_scalars on DVE (~310ns each). Steady state is ~99% tensor-busy at
the bf16 streaming floor (MLP 4 cyc/node + scatter 2 cyc/node); DMA
active ~160us (x1 bf16 + xT fp8 e3m4) fits underneath. Scatters run ~6
subs behind their exp so one-hot delivery from DVE never paces the PE.
Measured ~205us (baseline 252us) on fast-clock runs; chip DVFS state
varies run-to-run by up to ~20%.

Key dtype choices:
  - xT (score-MLP copy of x) is fp8 e3m4; W1 stays bf16 (mixed-operand
    matmul runs at full 1 cyc/row). Halves that stream's DMA. Measured
    end-to-end rel-err 0.90% vs the 2e-2 budget (e4m3 would be 2.7%).
  - x1 (scatter copy) stays bf16: scatter feeds the output directly.
  - exp weights round through bf16 in the one-hot; the host denominator
    uses the identical bf16 values, so no extra normalization error.
"""

import os
from contextlib import ExitStack

import ml_dtypes
import numpy as np

import concourse.bass as bass
import concourse.tile as tile
from concourse import bacc, mybir
from concourse.bass_utils import run_bass_kernel_spmd

NCORES = 8
G = 4096
H = 256
P = 128
F32 = mybir.dt.float32
BF16 = mybir.dt.bfloat16
FP8 = mybir.dt.float8e3
TANH = mybir.ActivationFunctionType.Tanh
EXP = mybir.ActivationFunctionType.Exp
BF = ml_dtypes.bfloat16
E3M4 = ml_dtypes.float8_e3m4

_prog_cache: dict[tuple, object] = {}
last_exec_time_ns = None
last_results = None


def _build_program(C: int, NWIN: int, b1_zero: bool = True):
    CT = C // P            # node tiles per window (multiple of 4)
    NG = -(-CT // 16)      # 2048-node DMA groups per window (last may be partial)
    nc = bacc.Bacc("TRN2")

    # x1[w, p, gt*256 + f] = x[node gt*128+p, f]   (node-major, bf16)
    x1 = nc.declare_dram_parameter("x1", [NWIN, P, CT * 256], BF16, isOutput=False)
    # xT[w, k, p, c] = x[node c, k*128+p]          (feature-major, fp8 e3m4)
    xT = nc.declare_dram_parameter("xT", [NWIN, 2, P, C], FP8, isOutput=False)
    # sbt[p, w*CT + gt] = local segment id of node gt*128+p in window w
    sbt = nc.declare_dram_parameter("sbt", [P, NWIN * CT], F32, isOutput=False)
    w1 = nc.declare_dram_parameter("w1", [2, P, 256], BF16, isOutput=False)
    w2b = nc.declare_dram_parameter("w2b", [P, 2], BF16, isOutput=False)
    b1b = nc.declare_dram_parameter("b1b", [P, 2], F32, isOutput=False)
    outp = nc.declare_dram_parameter("out", [NWIN, P, 256], F32, isOutput=True)
    wout = nc.declare_dram_parameter("wout", [NWIN, P, CT], F32, isOutput=True)

    with tile.TileContext(nc) as tc, ExitStack() as ctx:
        cpool = ctx.enter_context(tc.tile_pool(name="consts", bufs=1))
        xpool = ctx.enter_context(tc.tile_pool(name="xblk", bufs=7))
        xtpool = ctx.enter_context(tc.tile_pool(name="xtblk", bufs=6))
        h_ps = ctx.enter_context(tc.tile_pool(name="h_ps", bufs=3, space="PSUM"))
        e_ps = ctx.enter_context(tc.tile_pool(name="e_ps", bufs=1, space="PSUM"))
        w_ps = ctx.enter_context(tc.tile_pool(name="w_ps", bufs=1, space="PSUM"))
        h_sb = ctx.enter_context(tc.tile_pool(name="h_sb", bufs=4))
        wspool = ctx.enter_context(tc.tile_pool(name="wsw", bufs=3))
        ohpool = ctx.enter_context(tc.tile_pool(name="oh", bufs=36))
        opool = ctx.enter_context(tc.tile_pool(name="osb", bufs=2))

        w1t = cpool.tile([P, 2, 256], BF16, name="w1t")
        w2t = cpool.tile([P, 2], BF16, name="w2t")
        b1t = cpool.tile([P, 2], F32, name="b1t")
        iota = cpool.tile([P, P], BF16, name="iota")
        swt = cpool.tile([P, NWIN * CT], F32, name="swt")

        # on-device iota row (0..127 along free dim, same in every partition);
        # 0..127 are exact in bf16
        nc.gpsimd.iota(out=iota[:], pattern=[[1, P]], base=0,
                       channel_multiplier=0,
                       allow_small_or_imprecise_dtypes=True)

        def emit_const_dmas():
            # issued AFTER the first lead xts DMA so the MLP-critical input
            # leads the DMA queue; w1 next (the first LDWEIGHTS needs it)
            nc.sync.dma_start(out=w1t[:], in_=w1[:, :, :])

        def emit_const_dmas2():
            nc.sync.dma_start(out=w2t[:], in_=w2b[:])
            if not b1_zero:
                nc.sync.dma_start(out=b1t[:], in_=b1b[:])
            nc.sync.dma_start(out=swt[:], in_=sbt[:])

        # single PSUM tiles; groups/windows rotate through column halves so
        # WAR tracking provides double-buffering without extra banks
        wp_all = w_ps.tile([P, 32], F32, name="wp_all")
        ept_all = e_ps.tile([P, 512], F32, name="ept_all")

        wstate = {}

        micro_scores = []   # pending (ent, j, m) single score matmuls

        def push_sub_scores(ent):
            """queue one sub's score matmuls as micro-ops; they are drained
            one or two at a time between MLP/scatter streaming matmuls so
            their hs weight loads hide under the streams (the PE weight FIFO
            pipelines several LDWEIGHTS deep)."""
            for j in range(ent["ntiles"]):
                for m in range(2):
                    micro_scores.append((ent, j, m))
            ent["nmicro"] = 2 * ent["ntiles"]

        def emit_micro(n):
            for _ in range(n):
                if not micro_scores:
                    return
                ent, j, m = micro_scores.pop(0)
                hs, goff, hw = ent["hs"], ent["goff"], ent["hw"]
                c = goff + ent["coff"] + j
                nc.tensor.matmul(out=wp_all[:, c: c + 1],
                                 lhsT=hs[:, m * hw + j * 128: m * hw + (j + 1) * 128],
                                 rhs=w2t[:, m: m + 1],
                                 start=(m == 0), stop=(m == 1),
                                 skip_group_check=True)
                ent["nmicro"] -= 1

        def emit_score(ent):
            """fallback: emit one sub's scores contiguously (drain path)."""
            push_sub_scores(ent)
            emit_micro(2 * ent["ntiles"])

        def emit_oh(ent):
            """one-hot builds for one sub — emitted early on DVE."""
            w = ent["w"]
            st = wstate[w]
            ohs = []
            for j in range(ent["ntiles"]):
                gt = ent["gt0"] + j
                oh = ohpool.tile([P, P], BF16, name="oh")
                nc.vector.tensor_scalar(
                    out=oh[:], in0=iota[:],
                    scalar1=swt[:, w * CT + gt: w * CT + gt + 1],
                    scalar2=st["wsw"][:, gt: gt + 1],
                    op0=mybir.AluOpType.is_equal, op1=mybir.AluOpType.mult,
                )
                ohs.append(oh)
            ent["ohs"] = ohs

        def emit_scatter(ent):
            """scatter matmuls for one sub (4 tiles)."""
            w = ent["w"]
            x1t, xoff = ent["x1t"], ent["xoff"]
            woff = (w % 2) * 256
            for j in range(ent["ntiles"]):
                gt = ent["gt0"] + j
                nc.tensor.matmul(out=ept_all[:, woff:woff + 256],
                                 lhsT=ent["ohs"][j][:],
                                 rhs=x1t[:, (xoff + j) * 256:(xoff + j + 1) * 256],
                                 start=(gt == 0), stop=(gt == CT - 1),
                                 skip_group_check=True)
                emit_micro(1)
            if ent["last"]:
                ot = opool.tile([P, 256], F32, name="ot")
                nc.scalar.copy(ot[:], ept_all[:, woff:woff + 256])
                nc.sync.dma_start(out=outp[w], in_=ot[:])

        pend_score = []    # subs with hs written, score not yet emitted
        pend_exp = []      # blocks (1-2 subs) awaiting exp emission
        pend_scat = []     # subs whose exp is emitted, scatter not yet emitted

        def emit_exp(half_ent):
            """exp over one (1- or 2-sub) block: wp_all slice -> wsw slice."""
            w, goff = half_ent["w"], half_ent["goff"]
            subs = half_ent["subs"]
            wd = sum(s["ntiles"] for s in subs)
            col0 = subs[0]["gt0"]               # wsw column of first tile
            pcol0 = goff + subs[0]["coff"]      # wp_all column
            st = wstate[w]
            nc.scalar.activation(
                out=st["wsw"][:, col0:col0 + wd],
                in_=wp_all[:, pcol0: pcol0 + wd], func=EXP)
            st["nexp"] += wd
            if st["nexp"] >= CT:
                # all of this window's weights are final: export now (tail)
                nc.sync.dma_start(out=wout[w], in_=st["wsw"][:])
            for s in subs:
                emit_oh(s)
            pend_scat.extend(subs)

        for w in range(NWIN):
            wsw = wspool.tile([P, CT], F32, name="wsw")
            wstate[w] = {"wsw": wsw, "nexp": 0}
            for g in range(NG):
                nt = min(16, CT - g * 16)        # tiles in this group
                ns = nt // 4                     # subs in this group
                xts = xtpool.tile([P, 2, 2048], FP8, name="xts")
                if w == 0 and g == 0:
                    # lead chunk: first sub's nodes for both k halves, so the
                    # first MLP matmul can start as early as possible
                    nc.sync.dma_start(out=xts[:, :, :512],
                                      in_=xT[0, :, :, 0:512])
                    emit_const_dmas()
                    nc.sync.dma_start(out=xts[:, :, 512:nt * 128],
                                      in_=xT[0, :, :, 512:nt * 128])
                else:
                    nc.sync.dma_start(
                        out=xts[:, :, :nt * 128],
                        in_=xT[w, :, :, g * 2048:g * 2048 + nt * 128])
                x1t = xpool.tile([P, 16 * 256], BF16, name="x1t")
                nc.sync.dma_start(
                    out=x1t[:, :nt * 256],
                    in_=x1[w, :, g * 16 * 256:g * 16 * 256 + nt * 256])
                if w == 0 and g == 0:
                    emit_const_dmas2()
                goff = ((w * NG + g) % 2) * 16
                last_grp = (w == NWIN - 1 and g == NG - 1)
                subw = 2 if (last_grp or nt % 4 != 0) else 4  # tiles per sub
                nsub = nt // subw
                for subg in range(nsub):
                    toff = subg * subw           # tile offset within group
                    # start a new exp block on 8-tile boundaries; narrow
                    # (tail) subs each get their own block so the final
                    # serial chain is short
                    if subw == 2 or subg % 2 == 0:
                        half_ent = {"w": w, "goff": goff, "subs": []}
                        pend_exp.append(half_ent)
                        cur_ent = half_ent
                    else:
                        cur_ent = pend_exp[-1]
                    cw = subw * 128              # moving columns for this sub
                    hp = h_ps.tile([P, 2 * cw], F32, name="hp")
                    for m in range(2):
                        for k in range(2):
                            nc.tensor.matmul(
                                out=hp[:, m * cw:(m + 1) * cw],
                                lhsT=w1t[:, k, m * 128:(m + 1) * 128],
                                rhs=xts[:, k, toff * 128:toff * 128 + cw],
                                start=(k == 0), stop=(k == 1),
                            )
                            emit_micro(2)
                    hs = h_sb.tile([P, 2 * cw], BF16, name="hs")
                    if b1_zero:
                        nc.scalar.activation(out=hs[:], in_=hp[:], func=TANH)
                    else:
                        for m in range(2):
                            nc.scalar.activation(
                                out=hs[:, m * cw:(m + 1) * cw],
                                in_=hp[:, m * cw:(m + 1) * cw],
                                func=TANH, bias=b1t[:, m: m + 1])
                    gt0 = g * 16 + toff
                    sent = {"hs": hs, "goff": goff, "coff": toff, "w": w,
                            "gt0": gt0, "ntiles": subw, "hw": cw,
                            "x1t": x1t, "xoff": toff,
                            "last": gt0 + subw == CT}
                    cur_ent["subs"].append(sent)
                    pend_score.append(sent)
                    if len(pend_score) >= 3:
                        push_sub_scores(pend_score.pop(0))
                    # exp(block) once both its subs' scores are emitted
                    if pend_exp and all(
                            s.get("nmicro") == 0 for s in pend_exp[0]["subs"]):
                        emit_exp(pend_exp.pop(0))
                    lim = 3 if (w == NWIN - 1 and g >= NG - 2) else 6
                    if len(pend_scat) >= 5:
                        emit_scatter(pend_scat.pop(0))
                    while len(pend_scat) > lim:
                        emit_scatter(pend_scat.pop(0))
        while pend_score:
            emit_score(pend_score.pop(0))
        emit_micro(len(micro_scores))
        while pend_exp:
            emit_exp(pend_exp.pop(0))
        while pend_scat:
            emit_scatter(pend_scat.pop(0))
    nc.finalize()
    return nc


def _get_program(C: int, NWIN: int, b1_zero: bool = True):
    key = (C, NWIN, b1_zero)
    if key not in _prog_cache:
        _prog_cache[key] = _build_program(C, NWIN, b1_zero)
    return _prog_cache[key]


def kernel(x, batch, W1, b1, w2):
    global last_exec_time_ns, last_results
    x = np.ascontiguousarray(np.asarray(x, dtype=np.float32))
    batch_np = np.asarray(batch)
    batch_i = batch_np.astype(np.int64)
    W1 = np.asarray(W1, dtype=np.float32)
    b1 = np.asarray(b1, dtype=np.float32)
    w2 = np.asarray(w2, dtype=np.float32)
    N = x.shape[0]

    PC = -(-N // NCORES)               # nodes per core
    # pick NWIN so every equal-node window spans <= 128 segments
    NWIN = 4
    while True:
        WC = -(-PC // NWIN)            # real nodes per window
        ok = True
        for d in range(NCORES):
            for wloc in range(NWIN):
                ws = min(d * PC + wloc * WC, N)
                we = min(ws + WC, min((d + 1) * PC, N))
                if we > ws and int(batch_i[we - 1] - batch_i[ws]) >= P:
                    ok = False
                    break
            if not ok:
                break
        if ok:
            break
        NWIN += 1
    C = -(-WC // 256) * 256        # CT even; tail group uses 2-tile subs
    CT = C // P
    b1_zero = not bool(np.any(b1))
    nc = _get_program(C, NWIN, b1_zero=b1_zero)

    w1_dev = np.ascontiguousarray(W1.reshape(2, P, 256)).astype(BF)
    w2_dev = np.zeros((P, 2), np.float32)
    w2_dev[:, 0] = w2[:P, 0]
    w2_dev[:, 1] = w2[P:, 0]
    w2_dev = w2_dev.astype(BF)
    b1_dev = np.zeros((P, 2), np.float32)
    b1_dev[:, 0] = b1[:P]
    b1_dev[:, 1] = b1[P:]

    xbf = x.astype(BF)
    x8 = x.astype(E3M4)
    bases = np.zeros((NCORES, NWIN), np.int64)
    in_maps = []
    for d in range(NCORES):
        x1_dev = np.zeros((NWIN, P, CT * 256), BF)
        xT_dev = np.zeros((NWIN, 2, P, C), E3M4)
        sb_dev = np.empty((P, NWIN * CT), np.float32)
        for wloc in range(NWIN):
            ns = min(d * PC + wloc * WC, N)
            ne = min(ns + WC, min((d + 1) * PC, N))
            cnt = max(0, ne - ns)
            svals = np.full(C, -1.0, np.float32)
            if cnt:
                base = int(batch_i[ns])
                bases[d, wloc] = base
                xw = np.zeros((C, 256), BF)
                xw[:cnt] = xbf[ns:ne]
                x1_dev[wloc] = (
                    xw.reshape(CT, P, 256).transpose(1, 0, 2).reshape(P, CT * 256))
                xT_dev[wloc, 0, :, :cnt] = x8[ns:ne, :P].T
                xT_dev[wloc, 1, :, :cnt] = x8[ns:ne, P:].T
                svals[:cnt] = (batch_i[ns:ne] - base).astype(np.float32)
            sb_dev[:, wloc * CT:(wloc + 1) * CT] = svals.reshape(CT, P).T
        in_maps.append({"x1": x1_dev, "xT": xT_dev, "sbt": sb_dev, "w1": w1_dev,
                        "w2b": w2_dev, "b1b": b1_dev})

    res = run_bass_kernel_spmd(nc, in_maps, core_ids=list(range(NCORES)),
                               trace=bool(os.environ.get("KBENCH_TRACE")))
    last_exec_time_ns = res.exec_time_ns
    last_results = res

    E = np.zeros((G, H), np.float32)
    S = np.zeros((G,), np.float64)
    for d in range(NCORES):
        o = res.results[d]["out"]
        wso = res.results[d]["wout"]
        for wloc in range(NWIN):
            ns = min(d * PC + wloc * WC, N)
            ne = min(ns + WC, min((d + 1) * PC, N))
            cnt = max(0, ne - ns)
            if not cnt:
                continue
            base = int(bases[d, wloc])
            rmax = min(P, G - base)
            E[base:base + rmax] += o[wloc][:rmax]
            # node-order weights, rounded through bf16 exactly as the
            # scatter's bf16 one-hot used them
            wnode = (np.asarray(wso[wloc]).astype(np.float32)
                     .astype(BF).astype(np.float64).T.reshape(-1)[:cnt])
            segs = (batch_i[ns:ne] - base).astype(np.int64)
            S[base:base + rmax] += np.bincount(
                segs, weights=wnode, minlength=P)[:rmax]
    S = S.astype(np.float32)
    Ssafe = np.where(S == 0.0, 1.0, S)
    out = np.where((S > 0.0)[:, None], E / Ssafe[:, None], 0.0).astype(np.float32)
    return out
